# revision 1
# baseline (speedup 1.0000x reference)
"""Trainium2 Bass kernel for nn_MDRMWithCPRecon.

Sharding: pure data parallel over batch B=8 -> one batch element per
NeuronCore (8 cores). All parameters replicated. Each core computes the
full per-batch pipeline:

  x = cat(frm, oth)                 [512, 64, 64]
  Fm = lrelu(conv3x3(x, W3) + b3)   [256, 64, 64]   <- bulk of FLOPs
  U1/U2/U3 rank-4 softmax factors from pooled stats (tiny matmuls)
  spatial  = sigmoid(ws * U3 @ U2^T + bs)
  spectral = sigmoid(sigmoid(Wsa@mean + Wsm@max + biases))
  Wt = spectral x spatial
  fused    = a*Wt*frm + (1-a)*(1-Wt)*oth
  cp_recon = (Wr @ cp + br) * Wt + Fm,  cp = rank-4 CP(U1,U2,U3,lam)

Perf structure (vs the 294us first working version):
  - conv3x3 as 9-tap PSUM-accumulated f32r matmuls, weight-major: each
    512-row weight load feeds 4 consecutive matmuls into 4 parallel
    PSUM accumulators (the ISA caps the moving AP at 512 elements).
  - the last conv chunk runs tile-major with per-tile stats so the
    final pooled-stat reductions overlap its matmuls.
  - inputs DMAed straight into f32r SBUF tiles (f32r DRAM tensors);
    small weights batched into two blob DMAs.
  - Fm and the alpha-blend precomputation stay in SBUF as bf16.
  - pooled-stat -> U1/U2/U3 chain batched: one 384-column "a" matrix
    via rank-1 matmuls with bias rows, one batched 4-way softmax whose
    exp is a 4th-order Taylor series on the vector engine.
  - final stage: Wt folded as sigc = sigmoid(spatial)*spectral on ACT;
    products as plain bf16 tensor_tensor (2x DVE rate); the trailing
    "+ t2" / "+ Fm" adds are DMA-accumulates (software DGE) against
    baselines pre-written to DRAM during the conv.
  - outputs stored bf16 and widened to f32 on the host.
"""

import numpy as np

import concourse.bacc as bacc
import concourse.bass as bass
import concourse.tile as tile
from concourse import mybir, bass_utils

F32 = mybir.dt.float32
F32R = mybir.dt.float32r
BF16 = mybir.dt.bfloat16
AF = mybir.ActivationFunctionType
ALU = mybir.AluOpType
AX = mybir.AxisListType

B, C, H, W, K = 8, 256, 64, 64, 4
HW = H * W
NCORES = 8


def build_program(alpha, ws, bs):
    from concourse.masks import make_identity

    nc = bacc.Bacc("TRN2", target_bir_lowering=False, debug=False,
                   num_devices=NCORES)

    # ---- DRAM I/O (per core) ----
    xin_d = nc.dram_tensor("xin", [128, 4, 66, 66], F32R,
                           kind="ExternalInput")
    w3t_d = nc.dram_tensor("w3t", [128, 4, 9, 256], F32R, kind="ExternalInput")
    bb_d = nc.dram_tensor("bb", [128, 4], F32, kind="ExternalInput")
    wab_d = nc.dram_tensor("wab", [1, 9, 256], BF16, kind="ExternalInput")
    wu_d = nc.dram_tensor("wu", [128, 2, 4], BF16, kind="ExternalInput")
    bu_d = nc.dram_tensor("bu", [1, 4], BF16, kind="ExternalInput")
    wrt_d = nc.dram_tensor("wrt", [128, 2, 2, 128], BF16,
                           kind="ExternalInput")
    br_d = nc.dram_tensor("br", [1, 256], F32R, kind="ExternalInput")
    onesd = nc.dram_tensor("onesd", [1, HW], F32R, kind="ExternalInput")
    wsc_d = nc.dram_tensor("wsc", [128, 4, 2, 128], BF16,
                           kind="ExternalInput")
    lam_d = nc.dram_tensor("lam", [4, 1], F32, kind="ExternalInput")
    fused_o = nc.dram_tensor("fused", [2, 128, H, W], BF16,
                             kind="ExternalOutput")
    cpr_o = nc.dram_tensor("cpr", [2, 128, H, W], BF16, kind="ExternalOutput")

    with tile.TileContext(nc) as tc:
        _build_tile(tc, nc, make_identity, locals(), alpha, ws, bs)
    nc.compile()
    return nc


def _build_tile(tc, nc, make_identity, T, alpha, ws, bs):
    xin_d, w3t_d = T["xin_d"], T["w3t_d"]
    bb_d, wab_d, wu_d, bu_d = T["bb_d"], T["wab_d"], T["wu_d"], T["bu_d"]
    onesd = T["onesd"]
    wrt_d, br_d, wsc_d, lam_d = T["wrt_d"], T["br_d"], T["wsc_d"], T["lam_d"]
    fused_o, cpr_o = T["fused_o"], T["cpr_o"]

    import contextlib
    ctx = contextlib.ExitStack()
    consts = ctx.enter_context(tc.tile_pool(name="consts", bufs=1))
    scoped = contextlib.ExitStack()
    stage = scoped.enter_context(tc.tile_pool(name="stage", bufs=1))
    ew = stage

    # ================= consts / layout =================
    ident = consts.tile([128, 128], F32)
    make_identity(nc, ident[:])
    identb = consts.tile([128, 128], BF16)
    nc.vector.tensor_copy(identb[:], ident[:])
    ones128 = consts.tile([128, 1], F32)
    nc.gpsimd.memset(ones128[:], 1.0)
    ones_row = consts.tile([1, 256], BF16)
    nc.gpsimd.memset(ones_row[:], 1.0)
    ones4r = consts.tile([4, 128], F32R)
    nc.gpsimd.memset(ones4r.bitcast(F32)[:], 1.0)

    # conv weights + host-padded image in f32r (contiguous DMAs)
    w3t_r = consts.tile([128, 4, 9, 256], F32R)
    xr = consts.tile([128, 4, 66, 66], F32R)
    xrf = xr.bitcast(F32)

    # ---- input DMAs: small kt0 gating pieces first ----
    nc.scalar.dma_start(w3t_r[:, 0, 0:3], w3t_d[:, 0, 0:3])
    nc.sync.dma_start(xr[:, 0, 0:12, :], xin_d[:, 0, 0:12, :])
    nc.scalar.dma_start(w3t_r[:, 0, 3:9], w3t_d[:, 0, 3:9])
    nc.sync.dma_start(xr[:, 0, 12:34, :], xin_d[:, 0, 12:34, :])
    for kt in range(1, 4):
        nc.scalar.dma_start(w3t_r[:, kt], w3t_d[:, kt])
        nc.sync.dma_start(xr[:, kt, 0:34, :], xin_d[:, kt, 0:34, :])
    for kt in range(4):
        nc.sync.dma_start(xr[:, kt, 34:66, :], xin_d[:, kt, 34:66, :])

    # ---- small weights: host-converted bf16, DMAed straight in ----
    bb_sb = consts.tile([128, 4], F32)
    nc.gpsimd.dma_start(bb_sb[:], bb_d[:])
    lam_sb = consts.tile([4, 1], F32)
    nc.gpsimd.dma_start(lam_sb[:], lam_d[:])
    wab_b = consts.tile([1, 9, 256], BF16)
    nc.gpsimd.dma_start(wab_b[:], wab_d[:])
    wu_b = consts.tile([128, 2, 4], BF16)
    nc.gpsimd.dma_start(wu_b[:], wu_d[:])
    bu_b = consts.tile([1, 4], BF16)
    nc.gpsimd.dma_start(bu_b[:], bu_d[:])
    wrt_b = consts.tile([128, 2, 2, 128], BF16)
    nc.gpsimd.dma_start(wrt_b[:], wrt_d[:])
    wsc_b = consts.tile([128, 4, 2, 128], BF16)
    nc.gpsimd.dma_start(wsc_b[:], wsc_d[:])
    b3_sb = bb_sb[:, 0:2]                          # f32 conv bias
    bsc_sb = bb_sb[:, 2:4]                         # f32 spectral bias

    # ---- persistent intermediates ----
    Fm = consts.tile([128, 2, HW], BF16)          # conv output, (ct, h*64+w)
    Pd = consts.tile([128, 2, HW], BF16)          # alpha*frm - (1-alpha)*oth
    t2 = consts.tile([128, 2, HW], BF16)          # (1-alpha)*oth
    sums1 = consts.tile([128, 2, 2, 4], F32)      # ACT accum per 512-tile
    S_sum = consts.tile([128, 4, 64], F32)        # (m2ct0,m2ct1,m3ct0,m3ct1)
    S_max = consts.tile([128, 4, 64], F32)
    # mode2 partials: slots 0,1=(ct0,h0/h1) 2=(ct1,h0) 3:7=(ct1,h1) tiles
    pp_s = consts.tile([128, 7, 64], BF16)
    pp_m = consts.tile([128, 7, 64], BF16)
    stack4 = consts.tile([128, 4], F32)           # [sum1 ct0/1 | max1 ct0/1]
    srows1 = consts.tile([1, 2, 256], BF16)       # mode1 avg/max rows
    srowsS = consts.tile([1, 4, 64], BF16)        # m2avg,m2max,m3avg,m3max
    a_sb = consts.tile([128, 2, 384], BF16)       # adapter feats, both ct
    U = consts.tile([128, 3, 4], BF16)            # softmaxed factors
    U1T = consts.tile([4, 2, 128], BF16)
    UWH = consts.tile([4, 128], BF16)             # 0:64=U2T(w) 64:128=U3T(h)
    G5 = consts.tile([5, HW], F32R)               # G[r,hw]; row4 = ones
    G5f = G5.bitcast(F32)
    MT5 = consts.tile([5, 256], F32R)             # (Wr U1 lam)^T; row4 = br
    nc.sync.dma_start(G5[4:5, :], onesd[:])
    nc.sync.dma_start(MT5[4:5, :], br_d[:])
    gag = consts.tile([128, 4], BF16)             # [ga ct0/1 | gm ct0/1]
    spectral = consts.tile([128, 2], F32)

    # ---- blend precompute (runs during early conv; t2 -> DRAM base) ----
    for c2 in range(2):
        nc.scalar.activation(t2[:, c2].rearrange("p (h w) -> p h w", h=64),
                             xrf[:, 2 + c2, 1:65, 1:65], AF.Copy,
                             scale=float(1.0 - alpha))
        nc.vector.scalar_tensor_tensor(
            Pd[:, c2].rearrange("p (h w) -> p h w", h=64),
            xrf[:, c2, 1:65, 1:65], float(alpha),
            t2[:, c2].rearrange("p (h w) -> p h w", h=64),
            op0=ALU.mult, op1=ALU.subtract)

    # ================= conv3x3 (PE) + streaming stats =================
    # chunks 0-2 weight-major (amortize LDWEIGHTS across 4 parallel PSUM
    # accumulators); chunk 3 tile-major so its stats overlap its matmuls
    CHUNKS = [(0, 0), (1, 0), (0, 1), (1, 1)]     # (ct, half); h0 halves first
    with tc.tile_pool(name="ps_conv", bufs=8, space="PSUM") as ps_conv:
        for ci, (ct, hf) in enumerate(CHUNKS):
            if ci < 3:
                pss = [ps_conv.tile([128, 512], F32, tag="conv",
                                    name=f"cv{ci}{p}") for p in range(4)]
                idx = 0
                for kt in range(4):
                    for t in range(9):
                        dy, dx = t // 3, t % 3
                        for pq in range(4):
                            r0 = hf * 32 + pq * 8 + dy
                            nc.tensor.matmul(
                                pss[pq][:],
                                w3t_r[:, kt, t, ct * 128:(ct + 1) * 128],
                                xr[:, kt, r0: r0 + 8, dx: dx + 64],
                                start=(idx == 0), stop=(idx == 35))
                        idx += 1
                for pq in range(4):
                    o0 = hf * 2048 + pq * 512
                    nc.scalar.activation(Fm[:, ct, o0:o0 + 512], pss[pq][:],
                                         AF.Prelu, bias=b3_sb[:, ct:ct + 1],
                                         alpha=0.01,
                                         accum_out=sums1[:, ct, hf, pq:pq + 1])
                    nc.sync.dma_start(
                        cpr_o[ct, :,
                              hf * 32 + pq * 8: hf * 32 + (pq + 1) * 8, :],
                        Fm[:, ct, o0:o0 + 512].rearrange("p (h w) -> p h w",
                                                         h=8))
                fm_sl = Fm[:, ct, hf * 2048:(hf + 1) * 2048]
                blk = fm_sl.rearrange("p (h w) -> p h w", h=32)
                blk_t = fm_sl.rearrange("p (h w) -> p w h", h=32)
                nc.vector.tensor_reduce(S_sum[:, 2 + ct,
                                              hf * 32:(hf + 1) * 32],
                                        blk, axis=AX.X, op=ALU.add)
                nc.vector.tensor_reduce(S_max[:, 2 + ct,
                                              hf * 32:(hf + 1) * 32],
                                        blk, axis=AX.X, op=ALU.max)
                slot = ct * 2 + hf
                with nc.allow_low_precision(reason="bf16 pooled partials; "
                                            "0.4% fine for attention gates"):
                    nc.vector.tensor_reduce(pp_s[:, slot, :], blk_t,
                                            axis=AX.X, op=ALU.add)
                nc.vector.tensor_reduce(pp_m[:, slot, :], blk_t,
                                        axis=AX.X, op=ALU.max)
            else:
                for pq in range(4):
                    ps = ps_conv.tile([128, 512], F32, tag="conv",
                                      name=f"cv3{pq}")
                    idx = 0
                    for kt in range(4):
                        for t in range(9):
                            dy, dx = t // 3, t % 3
                            r0 = hf * 32 + pq * 8 + dy
                            nc.tensor.matmul(
                                ps[:],
                                w3t_r[:, kt, t, ct * 128:(ct + 1) * 128],
                                xr[:, kt, r0: r0 + 8, dx: dx + 64],
                                start=(idx == 0), stop=(idx == 35))
                            idx += 1
                    o0 = hf * 2048 + pq * 512
                    nc.scalar.activation(Fm[:, ct, o0:o0 + 512], ps[:],
                                         AF.Prelu, bias=b3_sb[:, ct:ct + 1],
                                         alpha=0.01,
                                         accum_out=sums1[:, ct, hf, pq:pq + 1])
                    nc.sync.dma_start(
                        cpr_o[ct, :,
                              hf * 32 + pq * 8: hf * 32 + (pq + 1) * 8, :],
                        Fm[:, ct, o0:o0 + 512].rearrange("p (h w) -> p h w",
                                                         h=8))
                    sl = Fm[:, ct, o0:o0 + 512]
                    blk = sl.rearrange("p (h w) -> p h w", h=8)
                    blk_t = sl.rearrange("p (h w) -> p w h", h=8)
                    s0 = hf * 32 + pq * 8
                    nc.vector.tensor_reduce(S_sum[:, 2 + ct, s0:s0 + 8], blk,
                                            axis=AX.X, op=ALU.add)
                    nc.vector.tensor_reduce(S_max[:, 2 + ct, s0:s0 + 8], blk,
                                            axis=AX.X, op=ALU.max)
                    with nc.allow_low_precision(reason="bf16 pooled "
                                                "partials; 0.4% fine"):
                        nc.vector.tensor_reduce(pp_s[:, 3 + pq, :], blk_t,
                                                axis=AX.X, op=ALU.add)
                    nc.vector.tensor_reduce(pp_m[:, 3 + pq, :], blk_t,
                                            axis=AX.X, op=ALU.max)

    # ================= pooled-stat rows -> U factors =================
    with tc.tile_pool(name="ps_mid", bufs=1, space="PSUM") as ps_mid:
        sm = ps_mid.tile([128, 512], F32, tag="sm")


        t3 = ps_mid.tile([4, 3, 128], BF16, tag="t3")
        mt2 = ps_mid.tile([4, 2, 128], F32, tag="mt2")
        row = ps_mid.tile([1, 512], F32, tag="row")
        rowb = ps_mid.tile([1, 256], F32, tag="rowb")
        tp = ps_mid.tile([64, 2, 128], F32, tag="tp")
        aps0 = ps_mid.tile([128, 384], F32, tag="aps0")
        aps1 = ps_mid.tile([128, 384], F32, tag="aps1")

        # preload the sigmoid ACT table while the ACT queue is idle
        warm1 = ew.tile([1, 1], F32, tag="warm1")
        nc.scalar.activation(warm1[:], ones128[0:1, 0:1], AF.Sigmoid)

        # combine mode2 partials via strided reduces over slot groups
        ppsv = pp_s[:].rearrange("p s w -> p w s")
        ppmv = pp_m[:].rearrange("p s w -> p w s")
        nc.vector.tensor_reduce(S_sum[:, 0, :], ppsv[:, :, 0:2], axis=AX.X,
                                op=ALU.add)
        nc.vector.tensor_reduce(S_sum[:, 1, :], ppsv[:, :, 2:7], axis=AX.X,
                                op=ALU.add)
        nc.vector.tensor_reduce(S_max[:, 0, :], ppmv[:, :, 0:2], axis=AX.X,
                                op=ALU.max)
        nc.vector.tensor_reduce(S_max[:, 1, :], ppmv[:, :, 2:7], axis=AX.X,
                                op=ALU.max)
        nc.vector.tensor_reduce(stack4[:, 0:2],
                                sums1[:].rearrange("p a b c -> p a (b c)"),
                                axis=AX.X, op=ALU.add)
        nc.vector.tensor_reduce(stack4[:, 2:4], S_max[:, 2:4, :], axis=AX.X,
                                op=ALU.max)
        # mode1 rows via per-column transposes
        for j in range(4):
            nc.tensor.transpose(row[0:1, j * 128:(j + 1) * 128],
                                stack4[:, j:j + 1], ident[:])
        nc.scalar.copy(srows1[0:1, 0, :], row[0:1, 0:256])
        nc.scalar.copy(srows1[0:1, 1, :], row[0:1, 256:512])
        # mode2/3 avg rows: ct-pair add folded into PSUM accumulation
        nc.tensor.matmul(rowb[0:1, 0:64], ones128[:], S_sum[:, 0, :],
                         start=True, stop=False)
        nc.tensor.matmul(rowb[0:1, 0:64], ones128[:], S_sum[:, 1, :],
                         start=False, stop=True)
        nc.tensor.matmul(rowb[0:1, 64:128], ones128[:], S_sum[:, 2, :],
                         start=True, stop=False)
        nc.tensor.matmul(rowb[0:1, 64:128], ones128[:], S_sum[:, 3, :],
                         start=False, stop=True)
        nc.scalar.copy(srowsS[0:1, 0, :], rowb[0:1, 0:64])
        nc.scalar.copy(srowsS[0:1, 2, :], rowb[0:1, 64:128])
        # mode2/3 max rows: ct-combine, transpose, reduce, transpose back
        pmx = ew.tile([128, 64], F32, tag="pmx")
        nc.vector.tensor_tensor(pmx[:], S_max[:, 0, :], S_max[:, 1, :],
                                op=ALU.max)
        qmx = ew.tile([128, 64], F32, tag="qmx")
        nc.vector.tensor_tensor(qmx[:], S_max[:, 2, :], S_max[:, 3, :],
                                op=ALU.max)
        nc.tensor.transpose(tp[:, 0, :], pmx[:], ident[:])
        mxr2 = ew.tile([64, 1], F32, tag="mxr2")
        nc.vector.tensor_reduce(mxr2[:], tp[:, 0, :], axis=AX.X, op=ALU.max)
        nc.tensor.transpose(tp[:, 1, :], qmx[:], ident[:])
        mxr3 = ew.tile([64, 1], F32, tag="mxr3")
        nc.vector.tensor_reduce(mxr3[:], tp[:, 1, :], axis=AX.X, op=ALU.max)
        nc.tensor.transpose(rowb[0:1, 128:192], mxr2[:], ident[0:64, 0:64])
        nc.tensor.transpose(rowb[0:1, 192:256], mxr3[:], ident[0:64, 0:64])
        nc.scalar.copy(srowsS[0:1, 1, :], rowb[0:1, 128:192])
        nc.scalar.copy(srowsS[0:1, 3, :], rowb[0:1, 192:256])

        # a = per-mode 1x1 adapters + bias, all modes in one 384-col matrix
        segs = [(0, 256, (0, 1, 2), 256), (256, 320, (3, 4, 5), 64),
                (320, 384, (6, 7, 8), 64)]
        rhs_rows = {0: srows1[0:1, 0, :], 1: srows1[0:1, 1, :],
                    3: srowsS[0:1, 0, :], 4: srowsS[0:1, 1, :],
                    6: srowsS[0:1, 2, :], 7: srowsS[0:1, 3, :]}
        for ct, a_ps in ((0, aps0), (1, aps1)):
            for c0, c1, wrows, n in segs:
                for i, wr in enumerate(wrows):
                    rhs = rhs_rows.get(wr, ones_row[0:1, 0:n])
                    nc.tensor.matmul(a_ps[:, c0:c1],
                                     wab_b[0:1, wr, ct * 128:(ct + 1) * 128],
                                     rhs, start=(i == 0), stop=(i == 2))
            nc.scalar.copy(a_sb[:, ct, :], a_ps[:])

        # u (transposed): [pos, k] per 128-position chunk
        ut = sm[:, 0:12]
        for ch in range(3):
            sl = ut[:, ch * 4:(ch + 1) * 4]
            nc.tensor.matmul(sl, a_sb[:, 0, ch * 128:(ch + 1) * 128],
                             wu_b[:, 0, :], start=True, stop=False)
            nc.tensor.matmul(sl, a_sb[:, 1, ch * 128:(ch + 1) * 128],
                             wu_b[:, 1, :], start=False, stop=False)
            nc.tensor.matmul(sl, ones_row[0:1, 0:128], bu_b[:],
                             start=False, stop=True)

        # batched softmax over k; exp via 4th-order Taylor (|u| ~ 0.1)
        utv = ut.rearrange("p (c k) -> p c k", c=3)
        h1 = ew.tile([128, 3, 4], F32, tag="h1")
        h2 = ew.tile([128, 3, 4], F32, tag="h2")
        nc.vector.tensor_scalar(h1[:], utv, 0.25, 1.0, op0=ALU.mult,
                                op1=ALU.add)
        nc.vector.tensor_tensor(h2[:], h1[:], utv, op=ALU.mult)
        nc.vector.tensor_scalar(h1[:], h2[:], 1.0 / 3.0, 1.0, op0=ALU.mult,
                                op1=ALU.add)
        nc.vector.tensor_tensor(h2[:], h1[:], utv, op=ALU.mult)
        nc.vector.tensor_scalar(h1[:], h2[:], 0.5, 1.0, op0=ALU.mult,
                                op1=ALU.add)
        nc.vector.tensor_tensor(h2[:], h1[:], utv, op=ALU.mult)
        nc.vector.tensor_scalar(h1[:], h2[:], 1.0, 1.0, op0=ALU.mult,
                                op1=ALU.add)
        ssum = ew.tile([128, 3], F32, tag="ssum")
        nc.vector.tensor_reduce(ssum[:], h1[:], axis=AX.X, op=ALU.add)
        rcp = ew.tile([128, 3], F32, tag="rcp")
        nc.vector.reciprocal(rcp[:], ssum[:])
        nc.vector.tensor_tensor(U[:], h1[:],
                                rcp[:, :, None].broadcast_to([128, 3, 4]),
                                op=ALU.mult)

        # transpose U chunks to rows
        for ch in range(3):
            nc.tensor.transpose(t3[:, ch, :], U[:, ch, :], identb[:])
            dst = UWH[:] if ch == 2 else U1T[:, ch, :]
            nc.scalar.copy(dst, t3[:, ch, :])

        # spectral input stats: F_spe = U1 @ [U2;U3]^T, then mean/max
        for ct in range(2):
            fps = sm[:, 128 + ct * 128:256 + ct * 128]
            nc.tensor.matmul(fps, U1T[:, ct, :], UWH[:], start=True,
                             stop=True)
            with nc.allow_low_precision(reason="128-val mean into bf16 for "
                                        "double-sigmoid gate; 0.4% is fine"):
                nc.vector.tensor_reduce(gag[:, ct:ct + 1], fps, axis=AX.X,
                                        op=ALU.add)
            nc.vector.tensor_reduce(gag[:, 2 + ct:3 + ct], fps, axis=AX.X,
                                    op=ALU.max)

        # MT = (Wr @ U1 diag(lam))^T
        for mm in range(2):
            m_ps = sm[:, 16 + mm * 4:20 + mm * 4]
            for kk in range(2):
                nc.tensor.matmul(m_ps, wrt_b[:, kk, mm, :], U[:, kk, :],
                                 start=(kk == 0), stop=(kk == 1))
            m_sb = ew.tile([128, 4], F32, tag=f"msb{mm}")
            nc.scalar.copy(m_sb[:], m_ps)
            nc.tensor.transpose(mt2[:, mm, :], m_sb[:], ident[:])
            nc.vector.tensor_scalar(MT5[0:4, mm * 128:(mm + 1) * 128],
                                    mt2[:, mm, :], lam_sb[:], None,
                                    op0=ALU.mult)

        # G[r, h*64+w] = U3T[r,h] * U2T[r,w]
        nc.vector.tensor_tensor(
            G5[0:4, :].rearrange("p (h w) -> p h w", h=64),
            UWH[:, 64:128][:, :, None].broadcast_to([4, 64, 64]),
            UWH[:, 0:64][:, None, :].broadcast_to([4, 64, 64]),
            op=ALU.mult)

        # spectral attention (double sigmoid)
        for mm in range(2):
            sp_ps = sm[:, 24 + mm:25 + mm]
            for kk in range(4):
                nc.tensor.matmul(sp_ps, wsc_b[:, kk, mm, :],
                                 gag[:, kk:kk + 1], start=(kk == 0),
                                 stop=(kk == 3))
            stmp = ew.tile([128, 1], F32, tag=f"stmp{mm}")
            nc.scalar.activation(stmp[:], sp_ps, AF.Sigmoid,
                                 bias=bsc_sb[:, mm:mm + 1])
            nc.scalar.activation(spectral[:, mm:mm + 1], stmp[:], AF.Sigmoid)

    # ================= final elementwise stage =================
    # fused = Wt*Pd (+ t2 via DMA-accum); cp_recon = Wt*rc (+ Fm via accum)
    scoped.close()
    sigp = ctx.enter_context(tc.tile_pool(name="sigp", bufs=2))
    tailp = ctx.enter_context(tc.tile_pool(name="tailp", bufs=2))
    with tc.tile_pool(name="ps_tail", bufs=2, space="PSUM") as ps_tail:
        def spat_q(q):
            sp_ps = ps_tail.tile([128, 1024], F32, tag="sp", name=f"sp{q}")
            for hx in range(2):
                h0 = q * 1024 + hx * 512
                nc.tensor.matmul(sp_ps[:, hx * 512:(hx + 1) * 512], ones4r[:],
                                 G5[0:4, h0:h0 + 512], start=True, stop=True)
            return sp_ps

        def tail_q(q, sp_ps):
            sig = sigp.tile([128, 1024], BF16, tag="sig", name=f"sig{q}")
            nc.scalar.activation(sig[:], sp_ps[:], AF.Sigmoid,
                                 scale=float(ws), bias=float(bs))
            qs = slice(q * 1024, (q + 1) * 1024)
            for ct in range(2):
                rc = ps_tail.tile([128, 1024], F32, tag="rc",
                                  name=f"rc{q}{ct}")
                for hx in range(2):
                    h0 = q * 1024 + hx * 512
                    nc.tensor.matmul(rc[:, hx * 512:(hx + 1) * 512],
                                     MT5[:, ct * 128:(ct + 1) * 128],
                                     G5[:, h0:h0 + 512], start=True,
                                     stop=True)
                sigc = sigp.tile([128, 1024], BF16, tag="sigc",
                                 name=f"sigc{q}{ct}")
                nc.scalar.activation(sigc[:], sig[:], AF.Copy,
                                     scale=spectral[:, ct:ct + 1])
                A2 = tailp.tile([128, 16, 64], BF16, tag="A2",
                                name=f"A2{q}{ct}")
                nc.vector.tensor_tensor(
                    A2[:], Pd[:, ct, qs].rearrange("p (h w) -> p h w", h=16),
                    sigc[:].rearrange("p (h w) -> p h w", h=16), op=ALU.mult)
                fu = tailp.tile([128, 16, 64], BF16, tag="fu",
                                name=f"fu{q}{ct}")
                nc.vector.tensor_tensor(
                    fu[:], A2[:],
                    t2[:, ct, qs].rearrange("p (h w) -> p h w", h=16),
                    op=ALU.add)
                nc.sync.dma_start(fused_o[ct, :, q * 16:(q + 1) * 16, :],
                                  fu[:])
                r2 = tailp.tile([128, 16, 64], BF16, tag="r2",
                                name=f"r2{q}{ct}")
                nc.vector.tensor_tensor(
                    r2[:], rc[:].rearrange("p (h w) -> p h w", h=16),
                    sigc[:].rearrange("p (h w) -> p h w", h=16), op=ALU.mult)
                nc.gpsimd.dma_start(cpr_o[ct, :, q * 16:(q + 1) * 16, :],
                                    r2[:], accum_op=ALU.add)

        sps = [spat_q(0), spat_q(1)]
        tail_q(0, sps[0])
        sps.append(spat_q(2))
        tail_q(1, sps[1])
        sps.append(spat_q(3))
        tail_q(2, sps[2])
        tail_q(3, sps[3])
    ctx.close()


def _prep_weights(W3, b3, Wa1, ba1, Wa2, ba2, Wa3, ba3, Wu, bu, Wr, br,
                  Wsa, bsa, Wsm, bsm):
    f = np.float32
    # w3t[p, kt, t, co] = W3[co, kt*128+p, dy, dx]
    w3t = np.ascontiguousarray(
        W3.reshape(C, 4, 128, 9).transpose(2, 1, 3, 0)).astype(f)
    b3h = np.ascontiguousarray(b3.reshape(2, 128).T).astype(f)
    # adapter weight/bias rows; avg-pool scalings folded in
    wab = np.zeros((1, 9, 256), f)
    wab[0, 0] = Wa1[:, 0] / float(HW)
    wab[0, 1] = Wa1[:, 1]
    wab[0, 2] = ba1
    wab[0, 3] = Wa2[:, 0] / float(C * H)
    wab[0, 4] = Wa2[:, 1]
    wab[0, 5] = ba2
    wab[0, 6] = Wa3[:, 0] / float(C * W)
    wab[0, 7] = Wa3[:, 1]
    wab[0, 8] = ba3
    wu = np.ascontiguousarray(
        Wu.reshape(K, 2, 128).transpose(2, 1, 0)).astype(f)
    buh = bu.reshape(1, 4).astype(f)
    # wrt[p, kk, mm, m] = Wr[mm*128+m, kk*128+p]
    wrt = np.ascontiguousarray(
        Wr.reshape(2, 128, 2, 128).transpose(3, 2, 0, 1)).astype(f)
    brh = br.reshape(1, 256).astype(f)
    # wsc[p, kk, mm, m]: kk<2 -> Wsa/128 (mean folded), kk>=2 -> Wsm
    wsa_r = (Wsa / 128.0).reshape(2, 128, 2, 128).transpose(3, 2, 0, 1)
    wsm_r = Wsm.reshape(2, 128, 2, 128).transpose(3, 2, 0, 1)
    wsc = np.ascontiguousarray(
        np.concatenate([wsa_r, wsm_r], axis=1)).astype(f)
    bsc = np.ascontiguousarray((bsa + bsm).reshape(2, 128).T).astype(f)
    import ml_dtypes
    bf = ml_dtypes.bfloat16
    return dict(w3t=w3t, bb=np.ascontiguousarray(
                    np.concatenate([b3h, bsc], axis=1)),
                wab=wab.astype(bf), wu=wu.astype(bf), bu=buh.astype(bf),
                wrt=wrt.astype(bf), br=brh, wsc=wsc.astype(bf),
                onesd=np.ones((1, HW), f))


_CACHE = {}


def kernel(frm_feat, other_feat, W3, b3, Wa1, ba1, Wa2, ba2, Wa3, ba3,
           Wu, bu, Wr, br, ws, bs, Wsa, bsa, Wsm, bsm, alpha, lam,
           _trace=False, _tmpdir=None):
    frm_feat = np.asarray(frm_feat, np.float32)
    other_feat = np.asarray(other_feat, np.float32)
    key = (float(alpha), float(ws), float(bs))
    if key not in _CACHE:
        _CACHE[key] = build_program(float(alpha), float(ws), float(bs))
    nc = _CACHE[key]

    wd = _prep_weights(np.asarray(W3), np.asarray(b3), np.asarray(Wa1),
                       np.asarray(ba1), np.asarray(Wa2), np.asarray(ba2),
                       np.asarray(Wa3), np.asarray(ba3), np.asarray(Wu),
                       np.asarray(bu), np.asarray(Wr), np.asarray(br),
                       np.asarray(Wsa), np.asarray(bsa), np.asarray(Wsm),
                       np.asarray(bsm))
    wd["lam"] = np.asarray(lam, np.float32).reshape(4, 1)

    in_maps = []
    for b_i in range(NCORES):
        m = dict(wd)
        xin = np.zeros((128, 4, 66, 66), np.float32)
        xin[:, 0:2, 1:65, 1:65] = frm_feat[b_i].reshape(
            2, 128, 64, 64).transpose(1, 0, 2, 3)
        xin[:, 2:4, 1:65, 1:65] = other_feat[b_i].reshape(
            2, 128, 64, 64).transpose(1, 0, 2, 3)
        m["xin"] = xin
        in_maps.append(m)

    res = bass_utils.run_bass_kernel_spmd(
        nc, in_maps, core_ids=list(range(NCORES)), trace=_trace,
        tmpdir=_tmpdir)
    fused = np.stack([
        np.asarray(res.results[i]["fused"]).astype(np.float32).reshape(C, H, W)
        for i in range(NCORES)])
    cpr = np.stack([
        np.asarray(res.results[i]["cpr"]).astype(np.float32).reshape(C, H, W)
        for i in range(NCORES)])
    kernel._last_exec_time_ns = res.exec_time_ns
    kernel._last_results = res
    return fused, cpr



# revision 6
# speedup vs baseline: 1.0895x; 1.0895x over previous
"""Trainium2 Bass kernel for nn_MDRMWithCPRecon.

Sharding: pure data parallel over batch B=8 -> one batch element per
NeuronCore (8 cores). All parameters replicated. Each core computes the
full per-batch pipeline:

  x = cat(frm, oth)                 [512, 64, 64]
  Fm = lrelu(conv3x3(x, W3) + b3)   [256, 64, 64]   <- bulk of FLOPs
  U1/U2/U3 rank-4 softmax factors from pooled stats (tiny matmuls)
  spatial  = sigmoid(ws * U3 @ U2^T + bs)
  spectral = sigmoid(sigmoid(Wsa@mean + Wsm@max + biases))
  Wt = spectral x spatial
  fused    = a*Wt*frm + (1-a)*(1-Wt)*oth
  cp_recon = (Wr @ cp + br) * Wt + Fm,  cp = rank-4 CP(U1,U2,U3,lam)

Perf structure (vs the 218us f32r version):
  - conv3x3 in bf16 (inputs + weights converted on host): same 1 cyc/row
    matmul rate as f32r but fast-weight-load kicks in (f32r pays a
    4-byte weight load per matmul) and input DMA bytes halve.
  - conv as 9-tap PSUM-accumulated matmuls, weight-major: each weight
    load feeds 4 consecutive matmuls into 4 parallel PSUM accumulators.
  - last conv chunk runs tile-major with per-tile stats so the final
    pooled-stat reductions overlap its matmuls.
  - pooled-stat -> U1/U2/U3 chain batched; 4-way softmax exp via a
    4th-order Taylor series on the vector engine.
  - G5/MT5/tail matmuls all bf16; G5 outer-product write at 2x DVE rate
    and scheduled before the spectral chain so tail PE work starts early.
  - sigc = sig*spectral on DVE tensor_scalar (per-partition scale)
    instead of ACT copies.
  - cp_recon "+ Fm" computed in SBUF (DVE/gpsimd split) and stored with
    plain DMA -- the previous DMA-accumulate drained ~18us of tiny
    software-DGE descriptors after the last compute.
  - outputs stored bf16 and widened to f32 on the host.
"""

import numpy as np

import concourse.bacc as bacc
import concourse.bass as bass
import concourse.tile as tile
from concourse import mybir, bass_utils

F32 = mybir.dt.float32
BF16 = mybir.dt.bfloat16
AF = mybir.ActivationFunctionType
ALU = mybir.AluOpType
AX = mybir.AxisListType

B, C, H, W, K = 8, 256, 64, 64, 4
HW = H * W
NCORES = 8


def build_program(alpha, ws, bs):
    from concourse.masks import make_identity

    nc = bacc.Bacc("TRN2", target_bir_lowering=False, debug=False,
                   num_devices=NCORES)

    # ---- DRAM I/O (per core) ----
    xin_d = nc.dram_tensor("xin", [128, 4, 66, 66], BF16,
                           kind="ExternalInput")
    w3t_d = nc.dram_tensor("w3t", [128, 4, 9, 256], BF16,
                           kind="ExternalInput")
    bb_d = nc.dram_tensor("bb", [128, 4], F32, kind="ExternalInput")
    wab_d = nc.dram_tensor("wab", [1, 9, 256], BF16, kind="ExternalInput")
    wu_d = nc.dram_tensor("wu", [128, 2, 4], BF16, kind="ExternalInput")
    bu_d = nc.dram_tensor("bu", [1, 4], BF16, kind="ExternalInput")
    wrt_d = nc.dram_tensor("wrt", [128, 2, 2, 128], BF16,
                           kind="ExternalInput")
    br_d = nc.dram_tensor("br", [1, 256], BF16, kind="ExternalInput")
    wsc_d = nc.dram_tensor("wsc", [128, 4, 2, 128], BF16,
                           kind="ExternalInput")
    lam_d = nc.dram_tensor("lam", [4, 1], F32, kind="ExternalInput")
    onesd = nc.dram_tensor("onesd", [1, HW], BF16, kind="ExternalInput")
    fused_o = nc.dram_tensor("fused", [2, 128, H, W], BF16,
                             kind="ExternalOutput")
    cpr_o = nc.dram_tensor("cpr", [2, 128, H, W], BF16, kind="ExternalOutput")

    with tile.TileContext(nc) as tc:
        _build_tile(tc, nc, make_identity, locals(), alpha, ws, bs)
    nc.compile()
    return nc


def _build_tile(tc, nc, make_identity, T, alpha, ws, bs):
    xin_d, w3t_d = T["xin_d"], T["w3t_d"]
    bb_d, wab_d, wu_d, bu_d = T["bb_d"], T["wab_d"], T["wu_d"], T["bu_d"]
    wrt_d, br_d, wsc_d, lam_d = T["wrt_d"], T["br_d"], T["wsc_d"], T["lam_d"]
    onesd = T["onesd"]
    fused_o, cpr_o = T["fused_o"], T["cpr_o"]

    import contextlib
    ctx = contextlib.ExitStack()
    consts = ctx.enter_context(tc.tile_pool(name="consts", bufs=1))
    scoped = contextlib.ExitStack()
    stage = scoped.enter_context(tc.tile_pool(name="stage", bufs=1))
    ew = stage

    # conv weights + host-padded image in bf16 (contiguous DMAs)
    w3t_r = consts.tile([128, 4, 9, 256], BF16)
    xr = consts.tile([128, 4, 66, 66], BF16)

    # ---- input DMAs first: small kt0 gating pieces, then the rest ----
    nc.scalar.dma_start(w3t_r[:, 0, 0:3], w3t_d[:, 0, 0:3])
    nc.sync.dma_start(xr[:, 0, 0:12, :], xin_d[:, 0, 0:12, :])
    nc.scalar.dma_start(w3t_r[:, 0, 3:9], w3t_d[:, 0, 3:9])
    nc.sync.dma_start(xr[:, 0, 12:34, :], xin_d[:, 0, 12:34, :])
    for kt in range(1, 4):
        nc.scalar.dma_start(w3t_r[:, kt], w3t_d[:, kt])
        nc.sync.dma_start(xr[:, kt, 0:34, :], xin_d[:, kt, 0:34, :])
    for kt in range(4):
        nc.sync.dma_start(xr[:, kt, 34:66, :], xin_d[:, kt, 34:66, :])

    # ================= consts / layout =================
    ident = consts.tile([128, 128], F32)
    make_identity(nc, ident[:])
    identb = consts.tile([128, 128], BF16)
    nc.vector.tensor_copy(identb[:], ident[:])
    ones128 = consts.tile([128, 1], F32)
    nc.gpsimd.memset(ones128[:], 1.0)
    ones_row = consts.tile([1, 256], BF16)
    nc.gpsimd.memset(ones_row[:], 1.0)
    ones4b = consts.tile([4, 128], BF16)
    nc.gpsimd.memset(ones4b[:], 1.0)

    # ---- small weights: host-converted bf16, DMAed straight in ----
    bb_sb = consts.tile([128, 4], F32)
    nc.gpsimd.dma_start(bb_sb[:], bb_d[:])
    lam_sb = consts.tile([4, 1], F32)
    nc.gpsimd.dma_start(lam_sb[:], lam_d[:])
    wab_b = consts.tile([1, 9, 256], BF16)
    nc.gpsimd.dma_start(wab_b[:], wab_d[:])
    wu_b = consts.tile([128, 2, 4], BF16)
    nc.gpsimd.dma_start(wu_b[:], wu_d[:])
    bu_b = consts.tile([1, 4], BF16)
    nc.gpsimd.dma_start(bu_b[:], bu_d[:])
    wrt_b = consts.tile([128, 2, 2, 128], BF16)
    nc.gpsimd.dma_start(wrt_b[:], wrt_d[:])
    wsc_b = consts.tile([128, 4, 2, 128], BF16)
    nc.gpsimd.dma_start(wsc_b[:], wsc_d[:])
    b3_sb = bb_sb[:, 0:2]                          # f32 conv bias
    bsc_sb = bb_sb[:, 2:4]                         # f32 spectral bias

    # ---- persistent intermediates ----
    Fm = consts.tile([128, 2, HW], BF16)          # conv output, (ct, h*64+w)
    Pd = consts.tile([128, 2, HW], BF16)          # alpha*frm - (1-alpha)*oth
    t2 = consts.tile([128, 2, HW], BF16)          # (1-alpha)*oth
    sums1 = consts.tile([128, 2, 2, 4], F32)      # ACT accum per 512-tile
    S_sum = consts.tile([128, 4, 64], F32)        # (m2ct0,m2ct1,m3ct0,m3ct1)
    S_max = consts.tile([128, 4, 64], F32)
    # mode2 partials: slots 0,1=(ct0,h0/h1) 2=(ct1,h0) 3:7=(ct1,h1) tiles
    pp_s = consts.tile([128, 7, 64], BF16)
    pp_m = consts.tile([128, 7, 64], BF16)
    stack4 = consts.tile([128, 4], F32)           # [sum1 ct0/1 | max1 ct0/1]
    srows1 = consts.tile([1, 2, 256], BF16)       # mode1 avg/max rows
    srowsS = consts.tile([1, 4, 64], BF16)        # m2avg,m2max,m3avg,m3max
    a_sb = consts.tile([128, 2, 384], BF16)       # adapter feats, both ct
    U = consts.tile([128, 3, 4], BF16)            # softmaxed factors
    U1T = consts.tile([4, 2, 128], BF16)
    UWH = consts.tile([4, 128], BF16)             # 0:64=U2T(w) 64:128=U3T(h)
    G5 = consts.tile([5, HW], BF16)               # G[r,hw]; row4 = ones
    nc.sync.dma_start(G5[4:5, :], onesd[:])
    MT5 = consts.tile([5, 256], BF16)             # (Wr U1 lam)^T; row4 = br
    nc.sync.dma_start(MT5[4:5, :], br_d[:])
    gag = consts.tile([128, 4], BF16)             # [ga ct0/1 | gm ct0/1]
    spectral = consts.tile([128, 2], F32)

    # ---- blend precompute (runs during early conv) ----
    for c2 in range(2):
        nc.scalar.activation(t2[:, c2].rearrange("p (h w) -> p h w", h=64),
                             xr[:, 2 + c2, 1:65, 1:65], AF.Copy,
                             scale=float(1.0 - alpha))
        nc.vector.scalar_tensor_tensor(
            Pd[:, c2].rearrange("p (h w) -> p h w", h=64),
            xr[:, c2, 1:65, 1:65], float(alpha),
            t2[:, c2].rearrange("p (h w) -> p h w", h=64),
            op0=ALU.mult, op1=ALU.subtract)

    # ================= conv3x3 (PE) + streaming stats =================
    # chunks 0-2 weight-major (amortize weight loads across 4 parallel PSUM
    # accumulators); chunk 3 tile-major so its stats overlap its matmuls
    CHUNKS = [(0, 0), (1, 0), (0, 1), (1, 1)]     # (ct, half); h0 halves first
    with tc.tile_pool(name="ps_conv", bufs=8, space="PSUM") as ps_conv:
        for ci, (ct, hf) in enumerate(CHUNKS):
            if ci < 3:
                pss = [ps_conv.tile([128, 512], F32, tag="conv",
                                    name=f"cv{ci}{p}") for p in range(4)]
                idx = 0
                for kt in range(4):
                    for t in range(9):
                        dy, dx = t // 3, t % 3
                        for pq in range(4):
                            r0 = hf * 32 + pq * 8 + dy
                            nc.tensor.matmul(
                                pss[pq][:],
                                w3t_r[:, kt, t, ct * 128:(ct + 1) * 128],
                                xr[:, kt, r0: r0 + 8, dx: dx + 64],
                                start=(idx == 0), stop=(idx == 35))
                        idx += 1
                for pq in range(4):
                    o0 = hf * 2048 + pq * 512
                    nc.scalar.activation(Fm[:, ct, o0:o0 + 512], pss[pq][:],
                                         AF.Prelu, bias=b3_sb[:, ct:ct + 1],
                                         alpha=0.01,
                                         accum_out=sums1[:, ct, hf, pq:pq + 1])
                fm_sl = Fm[:, ct, hf * 2048:(hf + 1) * 2048]
                blk = fm_sl.rearrange("p (h w) -> p h w", h=32)
                blk_t = fm_sl.rearrange("p (h w) -> p w h", h=32)
                nc.vector.tensor_reduce(S_sum[:, 2 + ct,
                                              hf * 32:(hf + 1) * 32],
                                        blk, axis=AX.X, op=ALU.add)
                nc.vector.tensor_reduce(S_max[:, 2 + ct,
                                              hf * 32:(hf + 1) * 32],
                                        blk, axis=AX.X, op=ALU.max)
                slot = ct * 2 + hf
                with nc.allow_low_precision(reason="bf16 pooled partials; "
                                            "0.4% fine for attention gates"):
                    nc.vector.tensor_reduce(pp_s[:, slot, :], blk_t,
                                            axis=AX.X, op=ALU.add)
                nc.vector.tensor_reduce(pp_m[:, slot, :], blk_t,
                                        axis=AX.X, op=ALU.max)
            else:
                for pq in range(4):
                    ps = ps_conv.tile([128, 512], F32, tag="conv",
                                      name=f"cv3{pq}")
                    idx = 0
                    for kt in range(4):
                        for t in range(9):
                            dy, dx = t // 3, t % 3
                            r0 = hf * 32 + pq * 8 + dy
                            nc.tensor.matmul(
                                ps[:],
                                w3t_r[:, kt, t, ct * 128:(ct + 1) * 128],
                                xr[:, kt, r0: r0 + 8, dx: dx + 64],
                                start=(idx == 0), stop=(idx == 35))
                            idx += 1
                    o0 = hf * 2048 + pq * 512
                    nc.scalar.activation(Fm[:, ct, o0:o0 + 512], ps[:],
                                         AF.Prelu, bias=b3_sb[:, ct:ct + 1],
                                         alpha=0.01,
                                         accum_out=sums1[:, ct, hf, pq:pq + 1])
                    sl = Fm[:, ct, o0:o0 + 512]
                    blk = sl.rearrange("p (h w) -> p h w", h=8)
                    blk_t = sl.rearrange("p (h w) -> p w h", h=8)
                    s0 = hf * 32 + pq * 8
                    nc.vector.tensor_reduce(S_sum[:, 2 + ct, s0:s0 + 8], blk,
                                            axis=AX.X, op=ALU.add)
                    nc.vector.tensor_reduce(S_max[:, 2 + ct, s0:s0 + 8], blk,
                                            axis=AX.X, op=ALU.max)
                    with nc.allow_low_precision(reason="bf16 pooled "
                                                "partials; 0.4% fine"):
                        nc.vector.tensor_reduce(pp_s[:, 3 + pq, :], blk_t,
                                                axis=AX.X, op=ALU.add)
                    nc.vector.tensor_reduce(pp_m[:, 3 + pq, :], blk_t,
                                            axis=AX.X, op=ALU.max)

    # ================= pooled-stat rows -> U factors =================
    with tc.tile_pool(name="ps_mid", bufs=1, space="PSUM") as ps_mid:
        sm = ps_mid.tile([128, 512], F32, tag="sm")

        t3 = ps_mid.tile([4, 3, 128], BF16, tag="t3")
        mt2 = ps_mid.tile([4, 2, 128], F32, tag="mt2")
        row = ps_mid.tile([1, 512], F32, tag="row")
        rowb = ps_mid.tile([1, 256], F32, tag="rowb")
        tp = ps_mid.tile([64, 2, 128], F32, tag="tp")
        aps0 = ps_mid.tile([128, 384], F32, tag="aps0")
        aps1 = ps_mid.tile([128, 384], F32, tag="aps1")

        # preload the sigmoid ACT table while the ACT queue is idle
        warm1 = ew.tile([1, 1], F32, tag="warm1")
        nc.scalar.activation(warm1[:], ones128[0:1, 0:1], AF.Sigmoid)

        # combine mode2 partials via strided reduces over slot groups
        ppsv = pp_s[:].rearrange("p s w -> p w s")
        ppmv = pp_m[:].rearrange("p s w -> p w s")
        nc.vector.tensor_reduce(S_sum[:, 0, :], ppsv[:, :, 0:2], axis=AX.X,
                                op=ALU.add)
        nc.vector.tensor_reduce(S_sum[:, 1, :], ppsv[:, :, 2:7], axis=AX.X,
                                op=ALU.add)
        nc.vector.tensor_reduce(S_max[:, 0, :], ppmv[:, :, 0:2], axis=AX.X,
                                op=ALU.max)
        nc.vector.tensor_reduce(S_max[:, 1, :], ppmv[:, :, 2:7], axis=AX.X,
                                op=ALU.max)
        nc.vector.tensor_reduce(stack4[:, 0:2],
                                sums1[:].rearrange("p a b c -> p a (b c)"),
                                axis=AX.X, op=ALU.add)
        nc.vector.tensor_reduce(stack4[:, 2:4], S_max[:, 2:4, :], axis=AX.X,
                                op=ALU.max)
        # mode1 rows via per-column transposes
        for j in range(4):
            nc.tensor.transpose(row[0:1, j * 128:(j + 1) * 128],
                                stack4[:, j:j + 1], ident[:])
        nc.scalar.copy(srows1[0:1, 0, :], row[0:1, 0:256])
        nc.scalar.copy(srows1[0:1, 1, :], row[0:1, 256:512])
        # mode2/3 avg rows: ct-pair add folded into PSUM accumulation
        nc.tensor.matmul(rowb[0:1, 0:64], ones128[:], S_sum[:, 0, :],
                         start=True, stop=False)
        nc.tensor.matmul(rowb[0:1, 0:64], ones128[:], S_sum[:, 1, :],
                         start=False, stop=True)
        nc.tensor.matmul(rowb[0:1, 64:128], ones128[:], S_sum[:, 2, :],
                         start=True, stop=False)
        nc.tensor.matmul(rowb[0:1, 64:128], ones128[:], S_sum[:, 3, :],
                         start=False, stop=True)
        nc.scalar.copy(srowsS[0:1, 0, :], rowb[0:1, 0:64])
        nc.scalar.copy(srowsS[0:1, 2, :], rowb[0:1, 64:128])
        # mode2/3 max rows: ct-combine, transpose, reduce, transpose back
        pmx = ew.tile([128, 64], F32, tag="pmx")
        nc.vector.tensor_tensor(pmx[:], S_max[:, 0, :], S_max[:, 1, :],
                                op=ALU.max)
        qmx = ew.tile([128, 64], F32, tag="qmx")
        nc.vector.tensor_tensor(qmx[:], S_max[:, 2, :], S_max[:, 3, :],
                                op=ALU.max)
        nc.tensor.transpose(tp[:, 0, :], pmx[:], ident[:])
        mxr2 = ew.tile([64, 1], F32, tag="mxr2")
        nc.vector.tensor_reduce(mxr2[:], tp[:, 0, :], axis=AX.X, op=ALU.max)
        nc.tensor.transpose(tp[:, 1, :], qmx[:], ident[:])
        mxr3 = ew.tile([64, 1], F32, tag="mxr3")
        nc.vector.tensor_reduce(mxr3[:], tp[:, 1, :], axis=AX.X, op=ALU.max)
        nc.tensor.transpose(rowb[0:1, 128:192], mxr2[:], ident[0:64, 0:64])
        nc.tensor.transpose(rowb[0:1, 192:256], mxr3[:], ident[0:64, 0:64])
        nc.scalar.copy(srowsS[0:1, 1, :], rowb[0:1, 128:192])
        nc.scalar.copy(srowsS[0:1, 3, :], rowb[0:1, 192:256])

        # a = per-mode 1x1 adapters + bias, all modes in one 384-col matrix
        segs = [(0, 256, (0, 1, 2), 256), (256, 320, (3, 4, 5), 64),
                (320, 384, (6, 7, 8), 64)]
        rhs_rows = {0: srows1[0:1, 0, :], 1: srows1[0:1, 1, :],
                    3: srowsS[0:1, 0, :], 4: srowsS[0:1, 1, :],
                    6: srowsS[0:1, 2, :], 7: srowsS[0:1, 3, :]}
        for ct, a_ps in ((0, aps0), (1, aps1)):
            for c0, c1, wrows, n in segs:
                for i, wr in enumerate(wrows):
                    rhs = rhs_rows.get(wr, ones_row[0:1, 0:n])
                    nc.tensor.matmul(a_ps[:, c0:c1],
                                     wab_b[0:1, wr, ct * 128:(ct + 1) * 128],
                                     rhs, start=(i == 0), stop=(i == 2))
            nc.scalar.copy(a_sb[:, ct, :], a_ps[:])

        # u (transposed): [pos, k] per 128-position chunk
        ut = sm[:, 0:12]
        for ch in range(3):
            sl = ut[:, ch * 4:(ch + 1) * 4]
            nc.tensor.matmul(sl, a_sb[:, 0, ch * 128:(ch + 1) * 128],
                             wu_b[:, 0, :], start=True, stop=False)
            nc.tensor.matmul(sl, a_sb[:, 1, ch * 128:(ch + 1) * 128],
                             wu_b[:, 1, :], start=False, stop=False)
            nc.tensor.matmul(sl, ones_row[0:1, 0:128], bu_b[:],
                             start=False, stop=True)

        # batched softmax over k; exp via 4th-order Taylor (|u| ~ 0.1)
        utv = ut.rearrange("p (c k) -> p c k", c=3)
        h1 = ew.tile([128, 3, 4], F32, tag="h1")
        h2 = ew.tile([128, 3, 4], F32, tag="h2")
        nc.vector.tensor_scalar(h1[:], utv, 0.25, 1.0, op0=ALU.mult,
                                op1=ALU.add)
        nc.vector.tensor_tensor(h2[:], h1[:], utv, op=ALU.mult)
        nc.vector.tensor_scalar(h1[:], h2[:], 1.0 / 3.0, 1.0, op0=ALU.mult,
                                op1=ALU.add)
        nc.vector.tensor_tensor(h2[:], h1[:], utv, op=ALU.mult)
        nc.vector.tensor_scalar(h1[:], h2[:], 0.5, 1.0, op0=ALU.mult,
                                op1=ALU.add)
        nc.vector.tensor_tensor(h2[:], h1[:], utv, op=ALU.mult)
        nc.vector.tensor_scalar(h1[:], h2[:], 1.0, 1.0, op0=ALU.mult,
                                op1=ALU.add)
        ssum = ew.tile([128, 3], F32, tag="ssum")
        nc.vector.tensor_reduce(ssum[:], h1[:], axis=AX.X, op=ALU.add)
        rcp = ew.tile([128, 3], F32, tag="rcp")
        nc.vector.reciprocal(rcp[:], ssum[:])
        nc.vector.tensor_tensor(U[:], h1[:],
                                rcp[:, :, None].broadcast_to([128, 3, 4]),
                                op=ALU.mult)

        # transpose U chunks to rows
        for ch in range(3):
            nc.tensor.transpose(t3[:, ch, :], U[:, ch, :], identb[:])
            dst = UWH[:] if ch == 2 else U1T[:, ch, :]
            nc.scalar.copy(dst, t3[:, ch, :])

        # G[r, h*64+w] = U3T[r,h] * U2T[r,w]  (early: gates tail PE work)
        nc.vector.tensor_tensor(
            G5[0:4, :].rearrange("p (h w) -> p h w", h=64),
            UWH[:, 64:128][:, :, None].broadcast_to([4, 64, 64]),
            UWH[:, 0:64][:, None, :].broadcast_to([4, 64, 64]),
            op=ALU.mult)

        # spectral input stats: F_spe = U1 @ [U2;U3]^T, then mean/max
        for ct in range(2):
            fps = sm[:, 128 + ct * 128:256 + ct * 128]
            nc.tensor.matmul(fps, U1T[:, ct, :], UWH[:], start=True,
                             stop=True)
            with nc.allow_low_precision(reason="128-val mean into bf16 for "
                                        "double-sigmoid gate; 0.4% is fine"):
                nc.vector.tensor_reduce(gag[:, ct:ct + 1], fps, axis=AX.X,
                                        op=ALU.add)
            nc.vector.tensor_reduce(gag[:, 2 + ct:3 + ct], fps, axis=AX.X,
                                    op=ALU.max)

        # MT = (Wr @ U1 diag(lam))^T
        for mm in range(2):
            m_ps = sm[:, 16 + mm * 4:20 + mm * 4]
            for kk in range(2):
                nc.tensor.matmul(m_ps, wrt_b[:, kk, mm, :], U[:, kk, :],
                                 start=(kk == 0), stop=(kk == 1))
            m_sb = ew.tile([128, 4], F32, tag=f"msb{mm}")
            nc.scalar.copy(m_sb[:], m_ps)
            nc.tensor.transpose(mt2[:, mm, :], m_sb[:], ident[:])
            with nc.allow_low_precision(reason="bf16 CP factors; gates "
                                        "tolerate 0.4%"):
                nc.vector.tensor_scalar(MT5[0:4, mm * 128:(mm + 1) * 128],
                                        mt2[:, mm, :], lam_sb[:], None,
                                        op0=ALU.mult)

        # spectral attention (double sigmoid)
        for mm in range(2):
            sp_ps = sm[:, 24 + mm:25 + mm]
            for kk in range(4):
                nc.tensor.matmul(sp_ps, wsc_b[:, kk, mm, :],
                                 gag[:, kk:kk + 1], start=(kk == 0),
                                 stop=(kk == 3))
            stmp = ew.tile([128, 1], F32, tag=f"stmp{mm}")
            nc.scalar.activation(stmp[:], sp_ps, AF.Sigmoid,
                                 bias=bsc_sb[:, mm:mm + 1])
            nc.scalar.activation(spectral[:, mm:mm + 1], stmp[:], AF.Sigmoid)

    # ================= final elementwise stage =================
    # fused = Wt*Pd + t2 ; cp_recon = (rc)*Wt + Fm, all in SBUF, plain DMA
    scoped.close()
    sigp = ctx.enter_context(tc.tile_pool(name="sigp", bufs=2))
    tailp = ctx.enter_context(tc.tile_pool(name="tailp", bufs=2))
    with tc.tile_pool(name="ps_tail", bufs=2, space="PSUM") as ps_tail:
        def spat_q(q):
            sp_ps = ps_tail.tile([128, 1024], F32, tag="sp", name=f"sp{q}")
            for hx in range(2):
                h0 = q * 1024 + hx * 512
                nc.tensor.matmul(sp_ps[:, hx * 512:(hx + 1) * 512],
                                 ones4b[:], G5[0:4, h0:h0 + 512],
                                 start=True, stop=True)
            return sp_ps

        def tail_q(q, sp_ps):
            sig = sigp.tile([128, 1024], BF16, tag="sig", name=f"sig{q}")
            nc.scalar.activation(sig[:], sp_ps[:], AF.Sigmoid,
                                 scale=float(ws), bias=float(bs))
            qs = slice(q * 1024, (q + 1) * 1024)
            for ct in range(2):
                rc = ps_tail.tile([128, 1024], F32, tag="rc",
                                  name=f"rc{q}{ct}")
                for hx in range(2):
                    h0 = q * 1024 + hx * 512
                    nc.tensor.matmul(rc[:, hx * 512:(hx + 1) * 512],
                                     MT5[:, ct * 128:(ct + 1) * 128],
                                     G5[:, h0:h0 + 512], start=True,
                                     stop=True)
                sigc = sigp.tile([128, 1024], BF16, tag="sigc",
                                 name=f"sigc{q}{ct}")
                nc.vector.tensor_scalar(sigc[:], sig[:],
                                        spectral[:, ct:ct + 1], None,
                                        op0=ALU.mult)
                A2 = tailp.tile([128, 16, 64], BF16, tag="A2",
                                name=f"A2{q}{ct}")
                nc.vector.tensor_tensor(
                    A2[:], Pd[:, ct, qs].rearrange("p (h w) -> p h w", h=16),
                    sigc[:].rearrange("p (h w) -> p h w", h=16), op=ALU.mult)
                fu = tailp.tile([128, 16, 64], BF16, tag="fu",
                                name=f"fu{q}{ct}")
                nc.vector.tensor_tensor(
                    fu[:], A2[:],
                    t2[:, ct, qs].rearrange("p (h w) -> p h w", h=16),
                    op=ALU.add)
                nc.sync.dma_start(fused_o[ct, :, q * 16:(q + 1) * 16, :],
                                  fu[:])
                r2 = tailp.tile([128, 16, 64], BF16, tag="r2",
                                name=f"r2{q}{ct}")
                nc.vector.tensor_tensor(
                    r2[:], rc[:].rearrange("p (h w) -> p h w", h=16),
                    sigc[:].rearrange("p (h w) -> p h w", h=16), op=ALU.mult)
                r3 = tailp.tile([128, 16, 64], BF16, tag="r3",
                                name=f"r3{q}{ct}")
                eng = nc.gpsimd if ct == 0 else nc.vector
                eng.tensor_tensor(
                    r3[:], r2[:],
                    Fm[:, ct, qs].rearrange("p (h w) -> p h w", h=16),
                    op=ALU.add)
                nc.scalar.dma_start(cpr_o[ct, :, q * 16:(q + 1) * 16, :],
                                    r3[:])

        sps = [spat_q(0), spat_q(1)]
        tail_q(0, sps[0])
        sps.append(spat_q(2))
        tail_q(1, sps[1])
        sps.append(spat_q(3))
        tail_q(2, sps[2])
        tail_q(3, sps[3])
    ctx.close()


def _prep_weights(W3, b3, Wa1, ba1, Wa2, ba2, Wa3, ba3, Wu, bu, Wr, br,
                  Wsa, bsa, Wsm, bsm):
    f = np.float32
    # w3t[p, kt, t, co] = W3[co, kt*128+p, dy, dx]
    w3t = np.ascontiguousarray(
        W3.reshape(C, 4, 128, 9).transpose(2, 1, 3, 0)).astype(f)
    b3h = np.ascontiguousarray(b3.reshape(2, 128).T).astype(f)
    # adapter weight/bias rows; avg-pool scalings folded in
    wab = np.zeros((1, 9, 256), f)
    wab[0, 0] = Wa1[:, 0] / float(HW)
    wab[0, 1] = Wa1[:, 1]
    wab[0, 2] = ba1
    wab[0, 3] = Wa2[:, 0] / float(C * H)
    wab[0, 4] = Wa2[:, 1]
    wab[0, 5] = ba2
    wab[0, 6] = Wa3[:, 0] / float(C * W)
    wab[0, 7] = Wa3[:, 1]
    wab[0, 8] = ba3
    wu = np.ascontiguousarray(
        Wu.reshape(K, 2, 128).transpose(2, 1, 0)).astype(f)
    buh = bu.reshape(1, 4).astype(f)
    # wrt[p, kk, mm, m] = Wr[mm*128+m, kk*128+p]
    wrt = np.ascontiguousarray(
        Wr.reshape(2, 128, 2, 128).transpose(3, 2, 0, 1)).astype(f)
    brh = br.reshape(1, 256).astype(f)
    # wsc[p, kk, mm, m]: kk<2 -> Wsa/128 (mean folded), kk>=2 -> Wsm
    wsa_r = (Wsa / 128.0).reshape(2, 128, 2, 128).transpose(3, 2, 0, 1)
    wsm_r = Wsm.reshape(2, 128, 2, 128).transpose(3, 2, 0, 1)
    wsc = np.ascontiguousarray(
        np.concatenate([wsa_r, wsm_r], axis=1)).astype(f)
    bsc = np.ascontiguousarray((bsa + bsm).reshape(2, 128).T).astype(f)
    import ml_dtypes
    bf = ml_dtypes.bfloat16
    return dict(w3t=w3t.astype(bf), bb=np.ascontiguousarray(
                    np.concatenate([b3h, bsc], axis=1)),
                wab=wab.astype(bf), wu=wu.astype(bf), bu=buh.astype(bf),
                wrt=wrt.astype(bf), br=brh.astype(bf), wsc=wsc.astype(bf),
                onesd=np.ones((1, HW), bf))


_CACHE = {}


def kernel(frm_feat, other_feat, W3, b3, Wa1, ba1, Wa2, ba2, Wa3, ba3,
           Wu, bu, Wr, br, ws, bs, Wsa, bsa, Wsm, bsm, alpha, lam,
           _trace=False, _tmpdir=None):
    import ml_dtypes
    bf = ml_dtypes.bfloat16
    frm_feat = np.asarray(frm_feat, np.float32)
    other_feat = np.asarray(other_feat, np.float32)
    key = (float(alpha), float(ws), float(bs))
    if key not in _CACHE:
        _CACHE[key] = build_program(float(alpha), float(ws), float(bs))
    nc = _CACHE[key]

    wd = _prep_weights(np.asarray(W3), np.asarray(b3), np.asarray(Wa1),
                       np.asarray(ba1), np.asarray(Wa2), np.asarray(ba2),
                       np.asarray(Wa3), np.asarray(ba3), np.asarray(Wu),
                       np.asarray(bu), np.asarray(Wr), np.asarray(br),
                       np.asarray(Wsa), np.asarray(bsa), np.asarray(Wsm),
                       np.asarray(bsm))
    wd["lam"] = np.asarray(lam, np.float32).reshape(4, 1)

    in_maps = []
    for b_i in range(NCORES):
        m = dict(wd)
        xin = np.zeros((128, 4, 66, 66), bf)
        xin[:, 0:2, 1:65, 1:65] = frm_feat[b_i].reshape(
            2, 128, 64, 64).transpose(1, 0, 2, 3).astype(bf)
        xin[:, 2:4, 1:65, 1:65] = other_feat[b_i].reshape(
            2, 128, 64, 64).transpose(1, 0, 2, 3).astype(bf)
        m["xin"] = xin
        in_maps.append(m)

    res = bass_utils.run_bass_kernel_spmd(
        nc, in_maps, core_ids=list(range(NCORES)), trace=_trace,
        tmpdir=_tmpdir)
    fused = np.stack([
        np.asarray(res.results[i]["fused"]).astype(np.float32).reshape(C, H, W)
        for i in range(NCORES)])
    cpr = np.stack([
        np.asarray(res.results[i]["cpr"]).astype(np.float32).reshape(C, H, W)
        for i in range(NCORES)])
    kernel._last_exec_time_ns = res.exec_time_ns
    kernel._last_results = res
    return fused, cpr
